# revision 4
# baseline (speedup 1.0000x reference)
"""CapsuleFC routing kernel for 8x Trainium2 NeuronCores.

Problem: B=64, N_IN=2048, D_IN=16, N_OUT=64, D_OUT=16
  votes  = einsum('bna,namd->bnmd', input, w)
  scores = einsum('bnmd,bmd->bnm', votes, ncv) / 4
  qk     = renorm(softmax(scores) * next_act)
  out    = einsum('bnm,bn,bnmd->bmd', qk, act, votes)

Sharding: tensor-parallel over the in-capsule axis N (n_shard = 256 per core).
qk is an n-sharded output (no collective); the bmd combine produces per-core
partials summed on the host.

Per-core dataflow (pairs of n mapped to the 128 partitions as (j, b)):
  PE   : votes via block-diag lhsT (K=32 = 2n x 16a), rhs = w slab streaming
  ACT  : PSUM->SBUF evacuation (copy), exp(scores) with fused accum S1
  DVE  : x ncv, d-reduce, softmax smalls, x (act/Z) x T
  PE   : combine = PAIRSUM.T @ wv accumulated over all pairs in PSUM
Free-dim layout is d-major: f = d*64 + m.
"""

import os
import sys

sys.path.insert(0, "/opt/trn_rl_repo")

import numpy as np

B, N, A, M, D = 64, 2048, 16, 64, 16
NCORES = 8
NS = N // NCORES          # 256 n per core
NPAIR = NS // 2           # 128 pairs per core
NBLK = NPAIR // 4         # 32 blocks of 4 pairs (8 n) per core
F = D * M                 # 1024, d-major
GATE_EPS = 1e-6
SCALE = 1.0 / np.sqrt(np.float32(D))  # 0.25
QCH = 16                  # qk staging chunk (pairs)

_cache = {}
last_results = None


def _build_nc():
    import concourse.bacc as bacc
    import concourse.tile as tile
    from concourse import mybir
    from contextlib import ExitStack

    f32 = mybir.dt.float32
    Alu = mybir.AluOpType
    Act = mybir.ActivationFunctionType

    nc = bacc.Bacc(
        "TRN2",
        target_bir_lowering=False,
        debug=False,
        enable_asserts=True,
        num_devices=NCORES,
    )

    wt = nc.dram_tensor("wt", [NBLK, 128, F], f32, kind="ExternalInput").ap()
    bdin = nc.dram_tensor("bdin", [NBLK, 128, 128], f32, kind="ExternalInput").ap()
    ncv2 = nc.dram_tensor("ncv2", [128, F], f32, kind="ExternalInput").ap()
    na2 = nc.dram_tensor("na2", [128, M], f32, kind="ExternalInput").ap()
    act2 = nc.dram_tensor("act2", [128, NPAIR], f32, kind="ExternalInput").ap()
    psid = nc.dram_tensor("psid", [128, B], f32, kind="ExternalInput").ap()
    qk_o = nc.dram_tensor("qk_o", [NPAIR, 128, M], f32, kind="ExternalOutput").ap()
    o2_o = nc.dram_tensor("o2_o", [B, F], f32, kind="ExternalOutput").ap()

    with ExitStack() as ctx:
        tc = ctx.enter_context(tile.TileContext(nc))
        singles = ctx.enter_context(tc.tile_pool(name="singles", bufs=1))
        wpool = ctx.enter_context(tc.tile_pool(name="wpool", bufs=3))
        bpool = ctx.enter_context(tc.tile_pool(name="bpool", bufs=3))
        vpool = ctx.enter_context(tc.tile_pool(name="vpool", bufs=3, space="PSUM"))
        opool = ctx.enter_context(tc.tile_pool(name="opool", bufs=1, space="PSUM"))
        evp = ctx.enter_context(tc.tile_pool(name="evp", bufs=3))
        wgt = ctx.enter_context(tc.tile_pool(name="wgt", bufs=2))
        wvp = ctx.enter_context(tc.tile_pool(name="wvp", bufs=2))
        small = ctx.enter_context(tc.tile_pool(name="small", bufs=6))
        qst = ctx.enter_context(tc.tile_pool(name="qst", bufs=2))

        ncv2_sb = singles.tile([128, F], f32)
        nc.sync.dma_start(out=ncv2_sb[:], in_=ncv2)
        na2_sb = singles.tile([128, M], f32)
        nc.sync.dma_start(out=na2_sb[:], in_=na2)
        act2_sb = singles.tile([128, NPAIR], f32)
        nc.sync.dma_start(out=act2_sb[:], in_=act2)
        psid_sb = singles.tile([128, B], f32)
        nc.sync.dma_start(out=psid_sb[:], in_=psid)

        o2_ps = opool.tile([B, F], f32)
        qs = None

        for blk in range(NBLK):
            wblk = wpool.tile([128, F], f32)
            nc.sync.dma_start(out=wblk[:], in_=wt[blk])
            bblk = bpool.tile([128, 128], f32)
            nc.sync.dma_start(out=bblk[:], in_=bdin[blk])
            for i in range(4):
                p = blk * 4 + i
                # --- votes: [128 = (2n, 64b), 1024 = (16d, 64m)] in PSUM
                v_ps = vpool.tile([128, F], f32)
                for h in range(2):
                    nc.tensor.matmul(
                        out=v_ps[:, h * 512 : (h + 1) * 512],
                        lhsT=bblk[32 * i : 32 * i + 32, :],
                        rhs=wblk[32 * i : 32 * i + 32, h * 512 : (h + 1) * 512],
                        start=True,
                        stop=True,
                        tile_position=(32 * i, 0),
                    )
                # --- evacuate PSUM -> SBUF on ScalarE
                votes_sb = evp.tile([128, F], f32)
                nc.scalar.copy(out=votes_sb[:], in_=v_ps[:])
                # --- weighted = votes * (ncv/4)
                wg = wgt.tile([128, F], f32)
                nc.vector.tensor_mul(wg[:], votes_sb[:], ncv2_sb[:])
                # --- scores[m] = sum_d weighted[(d,m)]
                scores = small.tile([128, M], f32)
                nc.vector.reduce_sum(
                    out=scores[:],
                    in_=wg[:].rearrange("q (d m) -> q m d", d=D),
                    axis=mybir.AxisListType.X,
                )
                # --- E = exp(scores), S1 = sum_m E
                e_t = small.tile([128, M], f32)
                s1 = small.tile([128, 1], f32)
                nc.scalar.activation(
                    out=e_t[:], in_=scores[:], func=Act.Exp, accum_out=s1[:]
                )
                # --- T = E * na, S2 = sum_m T
                t_t = small.tile([128, M], f32)
                s2 = small.tile([128, 1], f32)
                nc.vector.scalar_tensor_tensor(
                    out=t_t[:],
                    in0=e_t[:],
                    scalar=1.0,
                    in1=na2_sb[:],
                    op0=Alu.mult,
                    op1=Alu.mult,
                    accum_out=s2[:],
                )
                # --- Z = S2 + 1e-10*S1 ; R = 1/Z
                s2e = small.tile([128, 1], f32)
                nc.vector.tensor_scalar(
                    out=s2e[:],
                    in0=s1[:],
                    scalar1=1e-10,
                    scalar2=s2[:],
                    op0=Alu.mult,
                    op1=Alu.add,
                )
                r_t = small.tile([128, 1], f32)
                nc.vector.reciprocal(r_t[:], s2e[:])
                # --- qk = T * R  -> staging
                if p % QCH == 0:
                    qs = qst.tile([128, QCH, M], f32)
                nc.vector.tensor_scalar_mul(
                    out=qs[:, p % QCH, :], in0=t_t[:], scalar1=r_t[:]
                )
                # --- actR = act * R (per-partition combine coefficient)
                ar = small.tile([128, 1], f32)
                nc.vector.tensor_mul(ar[:], act2_sb[:, p : p + 1], r_t[:])
                # --- wv = votes * actR * T[m]
                wv = wvp.tile([128, D, M], f32)
                nc.vector.scalar_tensor_tensor(
                    out=wv[:],
                    in0=votes_sb[:].rearrange("q (d m) -> q d m", d=D),
                    scalar=ar[:],
                    in1=t_t[:].unsqueeze(1).broadcast_to([128, D, M]),
                    op0=Alu.mult,
                    op1=Alu.mult,
                )
                # --- combine: O2 += PAIRSUM.T @ wv  (PSUM accumulation)
                wv_f = wv[:].rearrange("q d m -> q (d m)")
                for h in range(2):
                    nc.tensor.matmul(
                        out=o2_ps[:, h * 512 : (h + 1) * 512],
                        lhsT=psid_sb[:],
                        rhs=wv_f[:, h * 512 : (h + 1) * 512],
                        start=(p == 0),
                        stop=(p == NPAIR - 1),
                        skip_group_check=True,
                    )
                # --- flush qk staging
                if p % QCH == QCH - 1:
                    p0 = p - (QCH - 1)
                    nc.sync.dma_start(
                        out=qk_o[p0 : p0 + QCH, :, :].transpose([1, 0, 2]),
                        in_=qs[:],
                    )
        # --- final: evacuate O2
        o2_sb = singles.tile([B, F], f32)
        nc.scalar.copy(out=o2_sb[:], in_=o2_ps[:])
        nc.sync.dma_start(out=o2_o, in_=o2_sb[:])

    nc.compile()
    return nc


def _prep_core_inputs(x_np, ca_np, ncv_np, na_np, w_np):
    """Build the 8 per-core input maps (host-side layout transforms)."""
    acts = np.clip(ca_np, GATE_EPS, 1.0 - GATE_EPS)

    ncv2 = (SCALE * ncv_np).transpose(0, 2, 1).reshape(B, F)  # [b, (d,m)]
    ncv2 = np.concatenate([ncv2, ncv2], axis=0).astype(np.float32)  # [128, F]
    na2 = np.concatenate([na_np, na_np], axis=0).astype(np.float32)  # [128, M]
    psid = np.zeros((128, B), np.float32)
    psid[np.arange(B), np.arange(B)] = 1.0
    psid[B + np.arange(B), np.arange(B)] = 1.0

    in_maps = []
    for c in range(NCORES):
        ns = slice(c * NS, (c + 1) * NS)
        wc = w_np[ns]  # [NS, A, M, D]
        wt_c = np.ascontiguousarray(wc.transpose(0, 1, 3, 2)).reshape(
            NBLK, 8 * A, F
        )  # [blk, (8n,16a), (d,m)]
        xT = np.ascontiguousarray(x_np[:, ns, :].transpose(1, 2, 0))  # [NS, A, B]
        xr = xT.reshape(NBLK, 4, 2, A, B)
        bd6 = np.zeros((NBLK, 4, 2, A, 2, B), np.float32)
        bd6[:, :, 0, :, 0, :] = xr[:, :, 0]
        bd6[:, :, 1, :, 1, :] = xr[:, :, 1]
        bdin_c = bd6.reshape(NBLK, 128, 128)
        act_c = (
            acts[:, ns].reshape(B, NPAIR, 2).transpose(2, 0, 1).reshape(128, NPAIR)
        )  # row j*64+b, col pair
        in_maps.append(
            {
                "wt": np.ascontiguousarray(wt_c, np.float32),
                "bdin": np.ascontiguousarray(bdin_c, np.float32),
                "ncv2": ncv2,
                "na2": na2,
                "act2": np.ascontiguousarray(act_c, np.float32),
                "psid": psid,
            }
        )
    return in_maps


def kernel(input, current_act, next_capsule_value, next_act, w, num_iter=1):
    global last_results
    from concourse.bass_utils import run_bass_kernel_spmd

    x_np = np.asarray(input, np.float32)
    ca_np = np.asarray(current_act, np.float32)
    ncv_np = np.asarray(next_capsule_value, np.float32)
    na_np = np.asarray(next_act, np.float32)
    w_np = np.asarray(w, np.float32)

    if "nc" not in _cache:
        _cache["nc"] = _build_nc()
    nc = _cache["nc"]

    in_maps = _prep_core_inputs(x_np, ca_np, ncv_np, na_np, w_np)

    trace = os.environ.get("BASS_TRACE", "").lower() in ("1", "true", "yes")
    import time as _time

    t0 = _time.time()
    res = run_bass_kernel_spmd(
        nc, in_maps, list(range(NCORES)), trace=trace
    )
    last_results = res
    globals()["last_run_wall_s"] = _time.time() - t0

    qk = np.empty((B, N, M), np.float32)
    o2sum = np.zeros((B, F), np.float64)
    for c, r in enumerate(res.results):
        q = r["qk_o"].reshape(NPAIR, 2, B, M).transpose(2, 0, 1, 3).reshape(B, NS, M)
        qk[:, c * NS : (c + 1) * NS, :] = q
        o2sum += r["o2_o"].astype(np.float64)
    out = (
        o2sum.reshape(B, D, M).transpose(0, 2, 1).astype(np.float32)
    )  # [B, M, D]
    return out, na_np.copy(), qk


# revision 8
# speedup vs baseline: 1.0457x; 1.0457x over previous
"""CapsuleFC routing kernel for 8x Trainium2 NeuronCores.

Problem: B=64, N_IN=2048, D_IN=16, N_OUT=64, D_OUT=16
  votes  = einsum('bna,namd->bnmd', input, w)
  scores = einsum('bnmd,bmd->bnm', votes, ncv) / 4
  qk     = renorm(softmax(scores) * next_act)
  out    = einsum('bnm,bn,bnmd->bmd', qk, act, votes)

Sharding: tensor-parallel over the in-capsule axis N (n_shard = 256 per core).
qk is an n-sharded output (no collective); the bmd combine produces per-core
partials summed on the host.

Per-core dataflow (pairs of n mapped to the 128 partitions as (j, b)):
  PE   : votes via block-diag lhsT (K=32 = 2n x 16a), rhs = w slab streaming
  ACT  : PSUM->SBUF evacuation (fp32->bf16), batched exp(scores)
  DVE  : x ncv (bf16 2x), reduce-tree lvl1, softmax smalls, wv = votes*actR*T
  GPSIMD: reduce-tree lvl2-4
  PE   : combine = PAIRSUM.T @ wv accumulated over all pairs in PSUM
Free-dim layout is d-major: f = d*64 + m.
"""

import os
import sys

sys.path.insert(0, "/opt/trn_rl_repo")

import numpy as np

B, N, A, M, D = 64, 2048, 16, 64, 16
NCORES = 8
NS = N // NCORES          # 256 n per core
NPAIR = NS // 2           # 128 pairs per core
NBLK = NPAIR // 4         # 32 blocks of 4 pairs (8 n) per core
F = D * M                 # 1024, d-major
GATE_EPS = 1e-6
SCALE = 1.0 / np.sqrt(np.float32(D))  # 0.25
QCH = 16                  # qk staging chunk (pairs)
EXPB = 4                  # exp batching (pairs)

_cache = {}
last_results = None
last_run_wall_s = None


def _bf16():
    import ml_dtypes

    return ml_dtypes.bfloat16


def _build_nc():
    import concourse.bacc as bacc
    import concourse.tile as tile
    from concourse import mybir
    from contextlib import ExitStack

    f32 = mybir.dt.float32
    bf16 = mybir.dt.bfloat16
    Alu = mybir.AluOpType
    Act = mybir.ActivationFunctionType

    nc = bacc.Bacc(
        "TRN2",
        target_bir_lowering=False,
        debug=False,
        enable_asserts=True,
        num_devices=NCORES,
    )

    wt = nc.dram_tensor("wt", [NBLK, 128, F], bf16, kind="ExternalInput").ap()
    bdin = nc.dram_tensor("bdin", [NBLK, 128, 128], bf16, kind="ExternalInput").ap()
    ncv2 = nc.dram_tensor("ncv2", [128, F], bf16, kind="ExternalInput").ap()
    na2 = nc.dram_tensor("na2", [128, M], bf16, kind="ExternalInput").ap()
    act2 = nc.dram_tensor("act2", [128, NPAIR], f32, kind="ExternalInput").ap()
    psid = nc.dram_tensor("psid", [128, B], bf16, kind="ExternalInput").ap()
    qk_o = nc.dram_tensor("qk_o", [NPAIR, 128, M], f32, kind="ExternalOutput").ap()
    o2_o = nc.dram_tensor("o2_o", [B, F], f32, kind="ExternalOutput").ap()

    with ExitStack() as ctx:
        tc = ctx.enter_context(tile.TileContext(nc))
        singles = ctx.enter_context(tc.tile_pool(name="singles", bufs=1))
        wpool = ctx.enter_context(tc.tile_pool(name="wpool", bufs=3))
        bpool = ctx.enter_context(tc.tile_pool(name="bpool", bufs=3))
        vpool = ctx.enter_context(tc.tile_pool(name="vpool", bufs=3, space="PSUM"))
        opool = ctx.enter_context(tc.tile_pool(name="opool", bufs=1, space="PSUM"))
        evp = ctx.enter_context(tc.tile_pool(name="evp", bufs=3))
        wgt = ctx.enter_context(tc.tile_pool(name="wgt", bufs=3))
        wvp = ctx.enter_context(tc.tile_pool(name="wvp", bufs=3))
        redp = ctx.enter_context(tc.tile_pool(name="redp", bufs=3))
        scq = ctx.enter_context(tc.tile_pool(name="scq", bufs=2))
        small = ctx.enter_context(tc.tile_pool(name="small", bufs=8))
        qst = ctx.enter_context(tc.tile_pool(name="qst", bufs=2))

        ncv2_sb = singles.tile([128, F], bf16)
        nc.sync.dma_start(out=ncv2_sb[:], in_=ncv2)
        na2_sb = singles.tile([128, M], bf16)
        nc.sync.dma_start(out=na2_sb[:], in_=na2)
        act2_sb = singles.tile([128, NPAIR], f32)
        nc.sync.dma_start(out=act2_sb[:], in_=act2)
        psid_sb = singles.tile([128, B], bf16)
        nc.sync.dma_start(out=psid_sb[:], in_=psid)

        o2_ps = opool.tile([B, F], f32)
        qs = None
        sc_q = None
        e_q = None

        for blk in range(NBLK):
            wblk = wpool.tile([128, F], bf16)
            nc.sync.dma_start(out=wblk[:], in_=wt[blk])
            bblk = bpool.tile([128, 128], bf16)
            nc.sync.dma_start(out=bblk[:], in_=bdin[blk])
            for i in range(4):
                p = blk * 4 + i
                v_ps = vpool.tile([128, F], f32)
                for h in range(2):
                    nc.tensor.matmul(
                        out=v_ps[:, h * 512 : (h + 1) * 512],
                        lhsT=bblk[32 * i : 32 * i + 32, :],
                        rhs=wblk[32 * i : 32 * i + 32, h * 512 : (h + 1) * 512],
                        start=True,
                        stop=True,
                        tile_position=(32 * i, 0),
                    )
                votes_sb = evp.tile([128, F], bf16)
                nc.scalar.copy(out=votes_sb[:], in_=v_ps[:])
                wg = wgt.tile([128, F], bf16)
                nc.vector.tensor_mul(wg[:], votes_sb[:], ncv2_sb[:])
                # d-reduce tree: 1024 -> 512 (DVE) -> 256 -> 128 -> 64 (GPSIMD)
                r1 = redp.tile([128, 512], bf16, tag="r1")
                nc.vector.tensor_add(r1[:], wg[:, 0:512], wg[:, 512:1024])
                r2 = redp.tile([128, 256], bf16, tag="r2")
                nc.gpsimd.tensor_add(r2[:], r1[:, 0:256], r1[:, 256:512])
                r3 = redp.tile([128, 128], bf16, tag="r3")
                nc.gpsimd.tensor_add(r3[:], r2[:, 0:128], r2[:, 128:256])
                scores = small.tile([128, M], bf16)
                nc.gpsimd.tensor_add(scores[:], r3[:, 0:64], r3[:, 64:128])
                e_t = small.tile([128, M], bf16)
                nc.scalar.activation(out=e_t[:], in_=scores[:], func=Act.Exp)
                t_t = small.tile([128, M], bf16)
                s2 = small.tile([128, 1], f32)
                nc.vector.scalar_tensor_tensor(
                    out=t_t[:],
                    in0=e_t[:],
                    scalar=1.0,
                    in1=na2_sb[:],
                    op0=Alu.mult,
                    op1=Alu.mult,
                    accum_out=s2[:],
                )
                r_t = small.tile([128, 1], f32)
                nc.vector.reciprocal(r_t[:], s2[:])
                # qk = T * R -> staging (fp32 out)
                if p % QCH == 0:
                    qs = qst.tile([128, QCH, M], f32)
                nc.vector.tensor_scalar_mul(
                    out=qs[:, p % QCH, :], in0=t_t[:], scalar1=r_t[:]
                )
                # actR = act * R
                ar = small.tile([128, 1], f32)
                nc.vector.tensor_mul(ar[:], act2_sb[:, p : p + 1], r_t[:])
                # wv = votes * actR * T[m]
                wv = wvp.tile([128, D, M], bf16)
                nc.vector.scalar_tensor_tensor(
                    out=wv[:],
                    in0=votes_sb[:].rearrange("q (d m) -> q d m", d=D),
                    scalar=ar[:],
                    in1=t_t[:].unsqueeze(1).broadcast_to([128, D, M]),
                    op0=Alu.mult,
                    op1=Alu.mult,
                )
                # combine: O2 += PAIRSUM.T @ wv (PSUM accumulation)
                wv_f = wv[:].rearrange("q d m -> q (d m)")
                for h in range(2):
                    nc.tensor.matmul(
                        out=o2_ps[:, h * 512 : (h + 1) * 512],
                        lhsT=psid_sb[:],
                        rhs=wv_f[:, h * 512 : (h + 1) * 512],
                        start=(p == 0),
                        stop=(p == NPAIR - 1),
                        skip_group_check=True,
                    )
                # flush qk staging
                if p % QCH == QCH - 1:
                    p0 = p - (QCH - 1)
                    nc.sync.dma_start(
                        out=qk_o[p0 : p0 + QCH, :, :].transpose([1, 0, 2]),
                        in_=qs[:],
                    )
        # final: evacuate O2
        o2_sb = singles.tile([B, F], f32)
        nc.scalar.copy(out=o2_sb[:], in_=o2_ps[:])
        nc.sync.dma_start(out=o2_o, in_=o2_sb[:])

    nc.compile()
    return nc


def _prep_core_inputs(x_np, ca_np, ncv_np, na_np, w_np):
    """Build the 8 per-core input maps (host-side layout transforms)."""
    bf = _bf16()
    acts = np.clip(ca_np, GATE_EPS, 1.0 - GATE_EPS)

    ncv2 = (SCALE * ncv_np).transpose(0, 2, 1).reshape(B, F)  # [b, (d,m)]
    ncv2 = np.concatenate([ncv2, ncv2], axis=0).astype(bf)  # [128, F]
    na2 = np.concatenate([na_np, na_np], axis=0).astype(bf)  # [128, M]
    psid = np.zeros((128, B), np.float32)
    psid[np.arange(B), np.arange(B)] = 1.0
    psid[B + np.arange(B), np.arange(B)] = 1.0
    psid = psid.astype(bf)

    in_maps = []
    for c in range(NCORES):
        ns = slice(c * NS, (c + 1) * NS)
        wc = w_np[ns]  # [NS, A, M, D]
        wt_c = np.ascontiguousarray(wc.transpose(0, 1, 3, 2)).reshape(
            NBLK, 8 * A, F
        )  # [blk, (8n,16a), (d,m)]
        xT = np.ascontiguousarray(x_np[:, ns, :].transpose(1, 2, 0))  # [NS, A, B]
        xr = xT.reshape(NBLK, 4, 2, A, B)
        bd6 = np.zeros((NBLK, 4, 2, A, 2, B), np.float32)
        bd6[:, :, 0, :, 0, :] = xr[:, :, 0]
        bd6[:, :, 1, :, 1, :] = xr[:, :, 1]
        bdin_c = bd6.reshape(NBLK, 128, 128)
        act_c = (
            acts[:, ns].reshape(B, NPAIR, 2).transpose(2, 0, 1).reshape(128, NPAIR)
        )  # row j*64+b, col pair
        in_maps.append(
            {
                "wt": np.ascontiguousarray(wt_c).astype(bf),
                "bdin": np.ascontiguousarray(bdin_c).astype(bf),
                "ncv2": ncv2,
                "na2": na2,
                "act2": np.ascontiguousarray(act_c, np.float32),
                "psid": psid,
            }
        )
    return in_maps


def kernel(input, current_act, next_capsule_value, next_act, w, num_iter=1):
    global last_results, last_run_wall_s
    from concourse.bass_utils import run_bass_kernel_spmd

    x_np = np.asarray(input, np.float32)
    ca_np = np.asarray(current_act, np.float32)
    ncv_np = np.asarray(next_capsule_value, np.float32)
    na_np = np.asarray(next_act, np.float32)
    w_np = np.asarray(w, np.float32)

    if "nc" not in _cache:
        _cache["nc"] = _build_nc()
    nc = _cache["nc"]

    in_maps = _prep_core_inputs(x_np, ca_np, ncv_np, na_np, w_np)

    trace = os.environ.get("BASS_TRACE", "").lower() in ("1", "true", "yes")
    import time as _time

    t0 = _time.time()
    res = run_bass_kernel_spmd(nc, in_maps, list(range(NCORES)), trace=trace)
    last_results = res
    last_run_wall_s = _time.time() - t0

    qk = np.empty((B, N, M), np.float32)
    o2sum = np.zeros((B, F), np.float64)
    for c, r in enumerate(res.results):
        q = r["qk_o"].reshape(NPAIR, 2, B, M).transpose(2, 0, 1, 3).reshape(B, NS, M)
        qk[:, c * NS : (c + 1) * NS, :] = q
        o2sum += r["o2_o"].astype(np.float64)
    out = o2sum.reshape(B, D, M).transpose(0, 2, 1).astype(np.float32)  # [B, M, D]
    return out, na_np.copy(), qk


# revision 9
# speedup vs baseline: 1.2230x; 1.1696x over previous
"""CapsuleFC routing kernel for 8x Trainium2 NeuronCores.

Problem: B=64, N_IN=2048, D_IN=16, N_OUT=64, D_OUT=16
  votes  = einsum('bna,namd->bnmd', input, w)
  scores = einsum('bnmd,bmd->bnm', votes, ncv) / 4
  qk     = renorm(softmax(scores) * next_act)
  out    = einsum('bnm,bn,bnmd->bmd', qk, act, votes)

Sharding: tensor-parallel over the in-capsule axis N (n_shard = 256 per core).
qk is an n-sharded output (no collective); the bmd combine produces per-core
partials summed on the host. qk normalization (T/S2) is done host-side.

Per-core dataflow (pairs of n mapped to the 128 partitions as (j, b)):
  PE    : votes via block-diag lhsT (K=32 = 2n x 16a), rhs = w slab streaming
  ACT   : PSUM->SBUF evacuation (fp32->bf16), block-batched exp(scores)
  DVE   : x ncv (bf16 2x), reduce lvl1, T=E*na (+S2 accum), CT=(T*R)*act, wv
  GPSIMD: reduce lvl2-4, block-batched, fp32 tail
  PE    : combine = PAIRSUM.T @ wv, deferred DEFER blocks to keep PE dense
Free-dim layout is d-major: f = d*64 + m.
"""

import os
import sys

sys.path.insert(0, "/opt/trn_rl_repo")

import numpy as np

B, N, A, M, D = 64, 2048, 16, 64, 16
NCORES = 8
NS = N // NCORES          # 256 n per core
NPAIR = NS // 2           # 128 pairs per core
NBLK = NPAIR // 4         # 32 blocks of 4 pairs (8 n) per core
F = D * M                 # 1024, d-major
GATE_EPS = 1e-6
SCALE = 1.0 / np.sqrt(np.float32(D))  # 0.25
QCH = 16                  # T staging chunk (pairs)
DEFER = 2                 # blocks of O2-matmul deferral

_cache = {}
last_results = None
last_run_wall_s = None


def _bf16():
    import ml_dtypes

    return ml_dtypes.bfloat16


def _build_nc():
    import concourse.bacc as bacc
    import concourse.tile as tile
    from concourse import mybir
    from contextlib import ExitStack

    f32 = mybir.dt.float32
    bf16 = mybir.dt.bfloat16
    Alu = mybir.AluOpType
    Act = mybir.ActivationFunctionType

    nc = bacc.Bacc(
        "TRN2",
        target_bir_lowering=False,
        debug=False,
        enable_asserts=True,
        num_devices=NCORES,
    )

    wt = nc.dram_tensor("wt", [NBLK, 128, F], bf16, kind="ExternalInput").ap()
    bdin = nc.dram_tensor("bdin", [NBLK, 128, 128], bf16, kind="ExternalInput").ap()
    ncv2 = nc.dram_tensor("ncv2", [128, F], bf16, kind="ExternalInput").ap()
    na2 = nc.dram_tensor("na2", [128, M], bf16, kind="ExternalInput").ap()
    act2 = nc.dram_tensor("act2", [128, NPAIR], f32, kind="ExternalInput").ap()
    psid = nc.dram_tensor("psid", [128, B], bf16, kind="ExternalInput").ap()
    t_o = nc.dram_tensor("t_o", [NPAIR, 128, M], f32, kind="ExternalOutput").ap()
    s2_o = nc.dram_tensor("s2_o", [128, NPAIR], f32, kind="ExternalOutput").ap()
    o2_o = nc.dram_tensor("o2_o", [B, F], f32, kind="ExternalOutput").ap()

    with ExitStack() as ctx:
        tc = ctx.enter_context(tile.TileContext(nc))
        singles = ctx.enter_context(tc.tile_pool(name="singles", bufs=1))
        wpool = ctx.enter_context(tc.tile_pool(name="wpool", bufs=3))
        bpool = ctx.enter_context(tc.tile_pool(name="bpool", bufs=3))
        vpool = ctx.enter_context(tc.tile_pool(name="vpool", bufs=3, space="PSUM"))
        opool = ctx.enter_context(tc.tile_pool(name="opool", bufs=1, space="PSUM"))
        evp = ctx.enter_context(tc.tile_pool(name="evp", bufs=4))
        wgt = ctx.enter_context(tc.tile_pool(name="wgt", bufs=3))
        wvp = ctx.enter_context(tc.tile_pool(name="wvp", bufs=4 * (DEFER + 2)))
        redp = ctx.enter_context(tc.tile_pool(name="redp", bufs=3))
        small = ctx.enter_context(tc.tile_pool(name="small", bufs=8))
        qst = ctx.enter_context(tc.tile_pool(name="qst", bufs=2))

        ncv2_sb = singles.tile([128, F], bf16)
        nc.sync.dma_start(out=ncv2_sb[:], in_=ncv2)
        na2_sb = singles.tile([128, M], bf16)
        nc.sync.dma_start(out=na2_sb[:], in_=na2)
        act2_sb = singles.tile([128, NPAIR], f32)
        nc.sync.dma_start(out=act2_sb[:], in_=act2)
        psid_sb = singles.tile([128, B], bf16)
        nc.sync.dma_start(out=psid_sb[:], in_=psid)
        s2_sb = singles.tile([128, NPAIR], f32)

        o2_ps = opool.tile([B, F], f32)
        ts = None
        wv_pend = []  # [(p, wv_tile)] awaiting deferred O2 matmuls

        def emit_o2(entries):
            for p, wv in entries:
                wv_f = wv[:].rearrange("q d m -> q (d m)")
                for h in range(2):
                    nc.tensor.matmul(
                        out=o2_ps[:, h * 512 : (h + 1) * 512],
                        lhsT=psid_sb[:],
                        rhs=wv_f[:, h * 512 : (h + 1) * 512],
                        start=(p == 0),
                        stop=(p == NPAIR - 1),
                        skip_group_check=True,
                    )

        for blk in range(NBLK):
            wblk = wpool.tile([128, F], bf16)
            nc.sync.dma_start(out=wblk[:], in_=wt[blk])
            bblk = bpool.tile([128, 128], bf16)
            nc.sync.dma_start(out=bblk[:], in_=bdin[blk])

            r1b = redp.tile([128, 4, 512], bf16, tag="r1b")
            r2b = redp.tile([128, 4, 256], f32, tag="r2b")
            r3b = redp.tile([128, 4, 128], f32, tag="r3b")
            scb = redp.tile([128, 4, M], f32, tag="scb")
            eb = redp.tile([128, 4, M], f32, tag="eb")
            votes_l = []

            # ---- phase A: votes + x ncv + reduce lvl1 (per pair)
            for i in range(4):
                v_ps = vpool.tile([128, F], f32)
                for h in range(2):
                    nc.tensor.matmul(
                        out=v_ps[:, h * 512 : (h + 1) * 512],
                        lhsT=bblk[32 * i : 32 * i + 32, :],
                        rhs=wblk[32 * i : 32 * i + 32, h * 512 : (h + 1) * 512],
                        start=True,
                        stop=True,
                        tile_position=(32 * i, 0),
                    )
                votes_sb = evp.tile([128, F], bf16)
                nc.scalar.copy(out=votes_sb[:], in_=v_ps[:])
                votes_l.append(votes_sb)
                wg = wgt.tile([128, F], bf16)
                nc.vector.tensor_mul(wg[:], votes_sb[:], ncv2_sb[:])
                nc.vector.tensor_add(r1b[:, i, :], wg[:, 0:512], wg[:, 512:1024])

            # ---- deferred combine for an older block (keeps PE dense)
            if blk >= DEFER:
                emit_o2(wv_pend[:4])
                del wv_pend[:4]

            # ---- phase B: block-batched reduce lvl2-4 (GPSIMD) + exp (ACT)
            nc.gpsimd.tensor_add(r2b[:], r1b[:, :, 0:256], r1b[:, :, 256:512])
            nc.gpsimd.tensor_add(r3b[:], r2b[:, :, 0:128], r2b[:, :, 128:256])
            nc.gpsimd.tensor_add(scb[:], r3b[:, :, 0:64], r3b[:, :, 64:128])
            nc.scalar.activation(out=eb[:], in_=scb[:], func=Act.Exp)

            # ---- phase C: per-pair softmax tail + wv
            for i in range(4):
                p = blk * 4 + i
                votes_sb = votes_l[i]
                if p % QCH == 0:
                    ts = qst.tile([128, QCH, M], f32)
                t_t = ts[:, p % QCH, :]
                nc.vector.scalar_tensor_tensor(
                    out=t_t,
                    in0=eb[:, i, :],
                    scalar=1.0,
                    in1=na2_sb[:],
                    op0=Alu.mult,
                    op1=Alu.mult,
                    accum_out=s2_sb[:, p : p + 1],
                )
                r_t = small.tile([128, 1], f32)
                nc.vector.reciprocal(r_t[:], s2_sb[:, p : p + 1])
                # CT = (T * R) * act  (combine coefficient, bf16)
                ct = small.tile([128, M], bf16)
                nc.vector.tensor_scalar(
                    out=ct[:],
                    in0=t_t,
                    scalar1=r_t[:],
                    scalar2=act2_sb[:, p : p + 1],
                    op0=Alu.mult,
                    op1=Alu.mult,
                )
                # wv = votes * CT[m]
                wv = wvp.tile([128, D, M], bf16)
                nc.vector.tensor_mul(
                    wv[:],
                    votes_sb[:].rearrange("q (d m) -> q d m", d=D),
                    ct[:].unsqueeze(1).broadcast_to([128, D, M]),
                )
                wv_pend.append((p, wv))
                # flush T staging
                if p % QCH == QCH - 1:
                    p0 = p - (QCH - 1)
                    nc.sync.dma_start(
                        out=t_o[p0 : p0 + QCH, :, :].transpose([1, 0, 2]),
                        in_=ts[:],
                    )
        # ---- drain deferred combines
        emit_o2(wv_pend)
        wv_pend.clear()

        nc.sync.dma_start(out=s2_o, in_=s2_sb[:])
        o2_sb = singles.tile([B, F], f32)
        nc.scalar.copy(out=o2_sb[:], in_=o2_ps[:])
        nc.sync.dma_start(out=o2_o, in_=o2_sb[:])

    nc.compile()
    return nc


def _prep_core_inputs(x_np, ca_np, ncv_np, na_np, w_np):
    """Build the 8 per-core input maps (host-side layout transforms)."""
    bf = _bf16()
    acts = np.clip(ca_np, GATE_EPS, 1.0 - GATE_EPS)

    ncv2 = (SCALE * ncv_np).transpose(0, 2, 1).reshape(B, F)  # [b, (d,m)]
    ncv2 = np.concatenate([ncv2, ncv2], axis=0).astype(bf)  # [128, F]
    na2 = np.concatenate([na_np, na_np], axis=0).astype(bf)  # [128, M]
    psid = np.zeros((128, B), np.float32)
    psid[np.arange(B), np.arange(B)] = 1.0
    psid[B + np.arange(B), np.arange(B)] = 1.0
    psid = psid.astype(bf)

    in_maps = []
    for c in range(NCORES):
        ns = slice(c * NS, (c + 1) * NS)
        wc = w_np[ns]  # [NS, A, M, D]
        wt_c = np.ascontiguousarray(wc.transpose(0, 1, 3, 2)).reshape(
            NBLK, 8 * A, F
        )  # [blk, (8n,16a), (d,m)]
        xT = np.ascontiguousarray(x_np[:, ns, :].transpose(1, 2, 0))  # [NS, A, B]
        xr = xT.reshape(NBLK, 4, 2, A, B)
        bd6 = np.zeros((NBLK, 4, 2, A, 2, B), np.float32)
        bd6[:, :, 0, :, 0, :] = xr[:, :, 0]
        bd6[:, :, 1, :, 1, :] = xr[:, :, 1]
        bdin_c = bd6.reshape(NBLK, 128, 128)
        act_c = (
            acts[:, ns].reshape(B, NPAIR, 2).transpose(2, 0, 1).reshape(128, NPAIR)
        )  # row j*64+b, col pair
        in_maps.append(
            {
                "wt": np.ascontiguousarray(wt_c).astype(bf),
                "bdin": np.ascontiguousarray(bdin_c).astype(bf),
                "ncv2": ncv2,
                "na2": na2,
                "act2": np.ascontiguousarray(act_c, np.float32),
                "psid": psid,
            }
        )
    return in_maps


def kernel(input, current_act, next_capsule_value, next_act, w, num_iter=1):
    global last_results, last_run_wall_s
    from concourse.bass_utils import run_bass_kernel_spmd

    x_np = np.asarray(input, np.float32)
    ca_np = np.asarray(current_act, np.float32)
    ncv_np = np.asarray(next_capsule_value, np.float32)
    na_np = np.asarray(next_act, np.float32)
    w_np = np.asarray(w, np.float32)

    if "nc" not in _cache:
        _cache["nc"] = _build_nc()
    nc = _cache["nc"]

    in_maps = _prep_core_inputs(x_np, ca_np, ncv_np, na_np, w_np)

    trace = os.environ.get("BASS_TRACE", "").lower() in ("1", "true", "yes")
    import time as _time

    t0 = _time.time()
    res = run_bass_kernel_spmd(nc, in_maps, list(range(NCORES)), trace=trace)
    last_results = res
    last_run_wall_s = _time.time() - t0

    qk = np.empty((B, N, M), np.float32)
    o2sum = np.zeros((B, F), np.float64)
    for c, r in enumerate(res.results):
        t = r["t_o"].reshape(NPAIR, 2, B, M)  # [pair, j, b, m]
        s2 = r["s2_o"].reshape(2, B, NPAIR)  # [j, b, pair]
        qc = t / s2.transpose(2, 0, 1)[:, :, :, None]
        qk[:, c * NS : (c + 1) * NS, :] = (
            qc.transpose(2, 0, 1, 3).reshape(B, NS, M)
        )
        o2sum += r["o2_o"].astype(np.float64)
    out = o2sum.reshape(B, D, M).transpose(0, 2, 1).astype(np.float32)  # [B, M, D]
    return out, na_np.copy(), qk
